# revision 17
# baseline (speedup 1.0000x reference)
"""Trainium2 Bass kernel for nn_Amodel_20933670600894 (ragged bi-GRU + MLP).

Data parallel over 8 cores (32 sequences each). Per core:
  Phase A: x1 = LayerNorm(series @ w_in + b_in)  -- LN done via centered
           weights (mean fold) + variance from a ones-matmul of squares;
           ln_g/ln_b folded into the gate matmul weights.
           gates_x = x1n @ (wi*ln_g).T + biases, with +30 bias folded into
           the z-gate wherever mask==0 so the time scan needs no mask.
           x_last (x1 at t=len-1) accumulated via a delta one-hot matmul.
  Phase B: 1024-step masked GRU scan, h kept as [128(h), 32(batch)] in SBUF.
  Phase C: backward GRU cell at last step, feature MLP, fusion head.
"""
import sys, os
sys.path.insert(0, "/opt/trn_rl_repo")

import numpy as np
import ml_dtypes
from contextlib import ExitStack

import concourse.bass as bass
import concourse.mybir as mybir
import concourse.tile as tile
from concourse import bacc
from concourse.bass_utils import run_bass_kernel_spmd

AF = mybir.ActivationFunctionType
ALU = mybir.AluOpType
F32 = mybir.dt.float32
BF16 = mybir.dt.bfloat16

B, T, SD, FD, H, NHID = 256, 1024, 64, 128, 128, 3
NCORES = 8
BS = B // NCORES          # 32 sequences per core
EPS = 1e-5
MASK_BIG = 30.0


def do_c_flag(p):
    return 'C' in p


def build(nc, T_=T, BS_=BS, CH_A=512, CH_S=64, phases='ABC'):
    """Build the per-core program. Token index = t*BS_ + b (t-major)."""
    NTOK = T_ * BS_
    CH_A = min(CH_A, NTOK, CH_S * BS_)
    n_tiles = NTOK // CH_A
    n_chunks = T_ // CH_S

    with tile.TileContext(nc) as tc:
        ctx = ExitStack()
        dram = ctx.enter_context(tc.tile_pool(name="dram", bufs=1, space="DRAM"))

        def din(name, shape):
            return dram.tile(shape, F32, kind="ExternalInput", name=name,
                             uniquify=False)

        series_t = dram.tile([SD, NTOK], BF16, kind="ExternalInput",
                              name="series_t", uniquify=False)
        mb_row = dram.tile([1, NTOK], BF16, kind="ExternalInput",
                            name="mb_row", uniquify=False)
        delta_row = dram.tile([1, NTOK], BF16, kind="ExternalInput",
                               name="delta_row", uniquify=False)
        w1_ext = din("w1_ext", [SD, H])              # W_centered
        b_ct = din("b_ct", [1, H])                   # b_centered
        wi_s = din("wi_s", [H, 3 * H])               # (wi * ln_g).T fwd
        bi_tot = din("bi_tot", [H, 3])               # per-gate bias totals fwd
        wh_t = din("wh_t", [H, 4 * H])               # [Wr,Wz,Wn,-Wz].T
        bhn = din("bhn", [H, 1])                     # bh_f n-slice
        wib_s = din("wib_s", [H, 3 * H])             # (wi_b * ln_g).T bwd
        bib_tot = din("bib_tot", [H, 3])             # per-gate bias totals bwd
        bhbn = din("bhbn", [H, 1])                   # bh_b n-slice
        feat_t = din("feat_t", [FD, BS_])            # feature transposed
        w0_t = din("w0_t", [FD, H])                  # feat_w0.T
        mlp_s = din("mlp_s", [H, NHID])              # bn scale per layer
        mlp_b = din("mlp_b", [H, NHID])              # bn shift per layer
        hw_t = din("hw_t", [H, (NHID - 1) * H])      # hid_w[i].T stacked
        o1_t = din("o1_t", [3 * H, H])               # out_w1.T
        ob1 = din("ob1", [H, 1])
        o2_t = din("o2_t", [H, H])                   # out_w2.T
        ob2 = din("ob2", [H, 1])
        o3_t = din("o3_t", [H, 1])                   # out_w3.T
        ob3 = din("ob3", [1, 1])
        out = dram.tile([1, BS_], F32, kind="ExternalOutput", name="out",
                        uniquify=False)


        const = ctx.enter_context(tc.tile_pool(name="const", bufs=1))
        # small constant tiles
        ones_div = const.tile([H, H], BF16)     # 1/H everywhere (var reduce)
        nc.vector.memset(ones_div[:], 1.0 / H)
        one_row = const.tile([1, H], BF16)      # broadcast row of ones
        nc.vector.memset(one_row[:], 1.0)
        eps_col = const.tile([H, 1], F32)
        nc.vector.memset(eps_col[:], EPS)

        _ld = [0]

        def load(pool, src, shape=None, name=None):
            _ld[0] += 1
            t_ = pool.tile(shape or src.shape, F32,
                           name=name or f"ld{_ld[0]}", tag=f"ldt{_ld[0]}")
            nc.sync.dma_start(t_[:], src[:])
            return t_

        def load_bf(pool, src, name):
            f32t = pool.tile(src.shape, F32, name=name + "_f", tag=name + "_f")
            nc.sync.dma_start(f32t[:], src[:])
            bft = pool.tile(src.shape, BF16, name=name, tag=name)
            nc.vector.tensor_copy(bft[:], f32t[:])
            return bft

        w1e_sb = load_bf(const, w1_ext, "w1e")  # [64, 128] bf16
        bct_sb = load_bf(const, b_ct, "bct")    # [1, 128] bf16
        wis_sb = load_bf(const, wi_s, "wis")    # [128, 384] bf16
        bit_sb = load(const, bi_tot)            # [128, 3]
        wht_sb = load_bf(const, wh_t, "wht")    # [128, 384] bf16
        bhn_sb = load(const, bhn)
        from concourse.masks import make_identity
        ident = const.tile([H, H], BF16, name="ident")
        make_identity(nc, ident[:])
        ones_ca = const.tile([1, CH_A], BF16, name="ones_ca")
        nc.vector.memset(ones_ca[:], 1.0)

        xacc = const.tile([H, CH_A], F32, name="xacc")
        nc.vector.memset(xacc[:], 0.0)

        # ---------------- Phases A+B interleaved: gate precompute feeds the
        # scan through an SBUF ring; Phase A work fills the scan's idle slots.
        ctx_a = ExitStack()
        pa = ctx_a.enter_context(tc.tile_pool(name="pa", bufs=2))
        pp_a = ctx_a.enter_context(tc.tile_pool(name="pp_a", bufs=1, space="PSUM"))
        pp_b = ctx_a.enter_context(tc.tile_pool(name="pp_b", bufs=1, space="PSUM"))
        pp_g = ctx_a.enter_context(tc.tile_pool(name="pp_g", bufs=1, space="PSUM"))
        pp_d = ctx_a.enter_context(tc.tile_pool(name="pp_d", bufs=1, space="PSUM"))
        ps = ctx_a.enter_context(tc.tile_pool(name="ps", bufs=2))
        pp_s = ctx_a.enter_context(tc.tile_pool(name="pp_s", bufs=2, space="PSUM"))

        h = const.tile([H, BS_], BF16, name="h")
        nc.vector.memset(h[:], 0.0)

        TPC = CH_S * BS_              # tokens per scan chunk
        apc = max(1, TPC // CH_A)     # A-tiles per scan chunk
        assert apc * CH_A == TPC or TPC < CH_A

        ring = []                     # (crz, cn) per chunk, pool-rotated

        def emit_a_chunk(c):
            """Phase A for scan chunk c: produce its gx ring tiles."""
            crz = ps.tile([H, CH_S * 3 * BS_], BF16, tag="crz")
            cn = ps.tile([H, CH_S * BS_], BF16, tag="cn")
            ring.append((crz, cn))
            for a in range(apc):
                i = c * apc + a
                S = slice(i * CH_A, (i + 1) * CH_A)
                s_t = pa.tile([SD, CH_A], BF16, tag="s_t")
                nc.sync.dma_start(s_t[:], series_t[:, S])
                mb_t = pa.tile([1, CH_A], BF16, tag="mb_t")
                nc.sync.dma_start(mb_t[:], mb_row[:, S])
                dl_t = pa.tile([1, CH_A], BF16, tag="dl_t")
                nc.sync.dma_start(dl_t[:], delta_row[:, S])
                x1c = pp_a.tile([H, CH_A], F32, tag="x1c")
                nc.tensor.matmul(x1c[:], w1e_sb[:], s_t[:], start=True, stop=False)
                nc.tensor.matmul(x1c[:], bct_sb[:], ones_ca[:], start=False,
                                 stop=True)
                x1s = pa.tile([H, CH_A], F32, tag="x1s")
                nc.vector.tensor_copy(x1s[:], x1c[:])
                sq = pa.tile([H, CH_A], BF16, tag="sq")
                nc.vector.tensor_mul(sq[:], x1s[:], x1s[:])
                var = pp_b.tile([H, CH_A], F32, tag="var")
                nc.tensor.matmul(var[:], ones_div[:], sq[:], start=True, stop=True)
                lnv = pa.tile([H, CH_A], F32, tag="lnv")
                nc.scalar.activation(lnv[:], var[:], AF.Ln, bias=eps_col[:, 0:1])
                rstd = pa.tile([H, CH_A], F32, tag="rstd")
                nc.scalar.activation(rstd[:], lnv[:], AF.Exp, scale=-0.5)
                x1n = pa.tile([H, CH_A], BF16, tag="x1n")
                nc.vector.tensor_mul(x1n[:], x1s[:], rstd[:])

                g_r = pp_g.tile([H, CH_A], F32, tag="g_r")
                g_z = pp_g.tile([H, CH_A], F32, tag="g_z")
                g_n = pp_g.tile([H, CH_A], F32, tag="g_n")
                nc.tensor.matmul(g_r[:], wis_sb[:, 0:H], x1n[:], start=True,
                                 stop=True)
                nc.tensor.matmul(g_z[:], wis_sb[:, H:2 * H], x1n[:], start=True,
                                 stop=False)
                nc.tensor.matmul(g_z[:], one_row[:], mb_t[:], start=False,
                                 stop=True)
                nc.tensor.matmul(g_n[:], wis_sb[:, 2 * H:3 * H], x1n[:],
                                 start=True, stop=True)
                nt = CH_A // BS_
                # evac straight into the ring tiles ([r,z,zneg] per step)
                rview = crz[:, 3 * a * CH_A:3 * (a + 1) * CH_A].rearrange(
                    "h (t three b) -> h (t three) b", three=3, b=BS_)
                dst_r = rview[:, 0::3, :]
                dst_z = rview[:, 1::3, :]
                dst_zn = rview[:, 2::3, :]
                nc.vector.tensor_scalar(dst_r, g_r[:].rearrange(
                    "h (t b) -> h t b", b=BS_), bit_sb[:, 0:1], None, op0=ALU.add)
                nc.vector.tensor_scalar(dst_z, g_z[:].rearrange(
                    "h (t b) -> h t b", b=BS_), bit_sb[:, 1:2], None, op0=ALU.add)
                nc.vector.tensor_scalar(dst_zn, g_z[:].rearrange(
                    "h (t b) -> h t b", b=BS_), bit_sb[:, 1:2], -1.0,
                    op0=ALU.add, op1=ALU.mult)
                nc.vector.tensor_scalar(cn[:, a * CH_A:(a + 1) * CH_A], g_n[:],
                                        bit_sb[:, 2:3], None, op0=ALU.add)

                db = pp_d.tile([H, CH_A], F32, tag="db")
                nc.tensor.matmul(db[:], one_row[:], dl_t[:], start=True,
                                 stop=True)
                tmp = pa.tile([H, CH_A], F32, tag="xtmp")
                nc.vector.tensor_mul(tmp[:], x1n[:], db[:])
                nc.vector.tensor_add(xacc[:], xacc[:], tmp[:])

        def emit_scan_chunk(c):
            crz, cn = ring[c]
            for j in range(CH_S):
                g = pp_s.tile([H, 4 * BS_], F32, tag="g")
                nc.tensor.matmul(g[:, 0:3 * BS_], ident[:],
                                 crz[:, j * 3 * BS_:(j + 1) * 3 * BS_],
                                 start=True, stop=False)
                nc.tensor.matmul(g[:, 0:BS_], wht_sb[:, 0:H], h[:],
                                 start=False, stop=True)
                nc.tensor.matmul(g[:, BS_:2 * BS_], wht_sb[:, H:2 * H], h[:],
                                 start=False, stop=True, skip_group_check=True)
                nc.tensor.matmul(g[:, 2 * BS_:3 * BS_], wht_sb[:, 3 * H:4 * H],
                                 h[:], start=False, stop=True,
                                 skip_group_check=True)
                nc.tensor.matmul(g[:, 3 * BS_:4 * BS_], wht_sb[:, 2 * H:3 * H],
                                 h[:], start=True, stop=True)
                rzz = ps.tile([H, 3 * BS_], F32, tag="rzz")
                nc.scalar.activation(rzz[:], g[:, 0:3 * BS_], AF.Sigmoid)
                u_ = ps.tile([H, BS_], F32, tag="u_")
                nc.gpsimd.tensor_mul(u_[:], rzz[:, BS_:2 * BS_], h[:])
                e2 = ps.tile([H, BS_], F32, tag="e2")
                nc.vector.scalar_tensor_tensor(
                    e2[:], g[:, 3 * BS_:4 * BS_], bhn_sb[:, 0:1], rzz[:, 0:BS_],
                    op0=ALU.add, op1=ALU.mult)
                t2 = ps.tile([H, BS_], F32, tag="t2")
                nc.vector.tensor_add(t2[:], e2[:],
                                     cn[:, j * BS_:(j + 1) * BS_])
                s_ = ps.tile([H, BS_], F32, tag="s_")
                nc.scalar.activation(s_[:], t2[:], AF.Sigmoid, scale=2.0)
                v_ = ps.tile([H, BS_], F32, tag="v_")
                nc.vector.scalar_tensor_tensor(v_[:], s_[:], 0.5,
                                               rzz[:, 2 * BS_:3 * BS_],
                                               op0=ALU.subtract, op1=ALU.mult)
                nc.vector.scalar_tensor_tensor(h[:], v_[:], 2.0, u_[:],
                                               op0=ALU.mult, op1=ALU.add)

        if 'A' in phases:
            emit_a_chunk(0)
            for c in range(n_chunks):
                if c + 1 < n_chunks:
                    emit_a_chunk(c + 1)
                if 'B' in phases:
                    emit_scan_chunk(c)

        # reduce xacc [H, CH_A] -> x_last [H, BS_] (tree over the t groups)
        width = CH_A if 'A' in phases else BS_
        while width > BS_:
            half = width // 2
            nc.vector.tensor_add(xacc[:, 0:half], xacc[:, 0:half],
                                 xacc[:, half:width])
            width = half
        x_last = xacc[:, 0:BS_]

        ctx_a.close()

        # ---------------- Phase C: backward cell, MLP, head ----------------
        pc = ctx.enter_context(tc.tile_pool(name="pc", bufs=1))
        pp_c = ctx.enter_context(tc.tile_pool(name="pp_c", bufs=1, space="PSUM"))
        wibs_sb = load_bf(pc, wib_s, "wibs")
        bibt_sb = load(pc, bib_tot)
        bhbn_sb = load(pc, bhbn)

        xl_bf = pc.tile([H, BS_], BF16, name="xl_bf")
        nc.vector.tensor_copy(xl_bf[:], x_last)
        gb = pp_c.tile([H, 3 * BS_], F32, tag="gb")
        for s in range(3):
            nc.tensor.matmul(gb[:, s * BS_:(s + 1) * BS_],
                             wibs_sb[:, s * H:(s + 1) * H], xl_bf[:],
                             start=True, stop=True)
        rb = pc.tile([H, BS_], F32, name="rb")
        nc.scalar.activation(rb[:], gb[:, 0:BS_], AF.Sigmoid,
                             bias=bibt_sb[:, 0:1])
        zb = pc.tile([H, BS_], F32, name="zb")
        nc.scalar.activation(zb[:], gb[:, BS_:2 * BS_], AF.Sigmoid,
                             bias=bibt_sb[:, 1:2])
        ub = pc.tile([H, BS_], F32, name="ub")
        nc.vector.tensor_scalar_mul(ub[:], rb[:], bhbn_sb[:, 0:1])
        tb = pc.tile([H, BS_], F32, name="tb")
        nc.vector.scalar_tensor_tensor(tb[:], gb[:, 2 * BS_:3 * BS_],
                                       bibt_sb[:, 2:3], ub[:],
                                       op0=ALU.add, op1=ALU.add)
        nb = pc.tile([H, BS_], F32, name="nb")
        nc.scalar.activation(nb[:], tb[:], AF.Tanh)
        vb = pc.tile([H, BS_], F32, name="vb")
        nc.vector.tensor_mul(vb[:], zb[:], nb[:])
        h_bwd = pc.tile([H, BS_], BF16, name="h_bwd")
        nc.vector.tensor_sub(h_bwd[:], nb[:], vb[:])

        # feature MLP
        featt_sb = load_bf(pc, feat_t, "featt")
        w0t_sb = load_bf(pc, w0_t, "w0t")
        mlps_sb = load(pc, mlp_s)
        mlpb_sb = load(pc, mlp_b)
        hwt_sb = load_bf(pc, hw_t, "hwt")
        x2 = featt_sb
        wts = [w0t_sb[:]] + [hwt_sb[:, i * H:(i + 1) * H] for i in range(NHID - 1)]
        for li in range(NHID):
            pm = pp_c.tile([H, BS_], F32, tag="pc")
            nc.tensor.matmul(pm[:], wts[li], x2[:], start=True, stop=True)
            x2n = pc.tile([H, BS_], BF16, name=f"x2_{li}")
            nc.scalar.activation(x2n[:], pm[:], AF.Lrelu,
                                 bias=mlpb_sb[:, li:li + 1],
                                 scale=mlps_sb[:, li:li + 1], alpha=0.01)
            x2 = x2n

        # head
        o1t_f = pc.tile([H, 3, H], F32, name="o1t_f")
        nc.sync.dma_start(o1t_f[:], o1_t[:])
        o1t_sb = pc.tile([H, 3, H], BF16, name="o1t")
        nc.vector.tensor_copy(o1t_sb[:], o1t_f[:])
        ob1_sb = load(pc, ob1)
        o2t_sb = load_bf(pc, o2_t, "o2t")
        ob2_sb = load(pc, ob2)
        o3t_sb = load_bf(pc, o3_t, "o3t")
        ob3_sb = load(pc, ob3)

        p1 = pp_c.tile([H, BS_], F32, tag="pc")
        nc.tensor.matmul(p1[:], o1t_sb[:, 0, :], h[:], start=True, stop=False)
        nc.tensor.matmul(p1[:], o1t_sb[:, 1, :], h_bwd[:], start=False, stop=False)
        nc.tensor.matmul(p1[:], o1t_sb[:, 2, :], x2[:], start=False, stop=True)
        y1 = pc.tile([H, BS_], BF16, name="y1")
        nc.scalar.activation(y1[:], p1[:], AF.Lrelu, bias=ob1_sb[:, 0:1],
                             alpha=0.01)
        p2 = pp_c.tile([H, BS_], F32, tag="pc")
        nc.tensor.matmul(p2[:], o2t_sb[:], y1[:], start=True, stop=True)
        y2 = pc.tile([H, BS_], BF16, name="y2")
        nc.scalar.activation(y2[:], p2[:], AF.Lrelu, bias=ob2_sb[:, 0:1],
                             alpha=0.01)
        p3 = pp_c.tile([1, BS_], F32, tag="pc3")
        nc.tensor.matmul(p3[:], o3t_sb[:], y2[:], start=True, stop=True)
        y3 = pc.tile([1, BS_], F32, name="y3")
        nc.scalar.activation(y3[:], p3[:], AF.Sigmoid, bias=ob3_sb[0:1, 0:1])
        nc.sync.dma_start(out[:], y3[:])

        ctx.close()
    nc.compile()
    return nc


def host_prep(inputs, BS_=BS):
    """Per-core input dicts (layout prep only)."""
    f = np.float32
    bs = inputs["batch_series"].astype(f)
    bm = inputs["batch_mask"].astype(f)
    bf = inputs["batch_feature"].astype(f)
    w_in, b_in = inputs["w_in"].astype(f), inputs["b_in"].astype(f)
    ln_g, ln_b = inputs["ln_g"].astype(f), inputs["ln_b"].astype(f)
    wi_f, wh_f = inputs["gru_wi_f"].astype(f), inputs["gru_wh_f"].astype(f)
    bi_f, bh_f = inputs["gru_bi_f"].astype(f), inputs["gru_bh_f"].astype(f)
    wi_b = inputs["gru_wi_b"].astype(f)
    bi_b, bh_b = inputs["gru_bi_b"].astype(f), inputs["gru_bh_b"].astype(f)

    # LN folds
    w_ct = (w_in - w_in.mean(0, keepdims=True)).T.copy()        # [SD, H]
    b_ct = (b_in - b_in.mean())[None, :]                        # [1, H]
    w1_ext = w_ct.astype(f)                                     # [SD, H]
    wi_s = (wi_f * ln_g[None, :]).T.copy().astype(f)            # [H, 3H]
    wib_s = (wi_b * ln_g[None, :]).T.copy().astype(f)
    lnb_f = wi_f @ ln_b                                          # [3H]
    lnb_b = wi_b @ ln_b
    bt_f = bi_f + lnb_f
    bt_f[0:2 * H] += bh_f[0:2 * H]
    bi_tot = np.stack([bt_f[0:H], bt_f[H:2 * H], bt_f[2 * H:3 * H]], 1).astype(f)
    bt_b = bi_b + lnb_b
    bt_b[0:2 * H] += bh_b[0:2 * H]
    bib_tot = np.stack([bt_b[0:H], bt_b[H:2 * H], bt_b[2 * H:3 * H]], 1).astype(f)

    bn_scale = 1.0 / np.sqrt(1.0 + EPS)
    mlp_s = np.stack([inputs["bn0_g"].astype(f) * bn_scale] +
                     [inputs["hbn_g"][i].astype(f) * bn_scale
                      for i in range(NHID - 1)], 1).astype(f)
    mlp_b = np.stack(
        [inputs["feat_b0"].astype(f) * bn_scale * inputs["bn0_g"].astype(f)
         + inputs["bn0_b"].astype(f)] +
        [inputs["hid_b"][i].astype(f) * bn_scale * inputs["hbn_g"][i].astype(f)
         + inputs["hbn_b"][i].astype(f) for i in range(NHID - 1)],
        1).astype(f)
    hw_t = np.concatenate([inputs["hid_w"][i].astype(f).T
                           for i in range(NHID - 1)], 1).astype(f)

    shared = dict(
        w1_ext=w1_ext, b_ct=np.ascontiguousarray(b_ct).astype(f), wi_s=wi_s,
        bi_tot=bi_tot,
        wh_t=np.concatenate([wh_f.T, -wh_f.T[:, H:2 * H]], 1).copy().astype(f),
        bhn=bh_f[2 * H:3 * H, None].astype(f),
        wib_s=wib_s, bib_tot=bib_tot,
        bhbn=bh_b[2 * H:3 * H, None].astype(f),
        w0_t=inputs["feat_w0"].astype(f).T.copy(),
        mlp_s=mlp_s, mlp_b=mlp_b, hw_t=hw_t,
        o1_t=np.ascontiguousarray(inputs["out_w1"].astype(f).T.reshape(3, H, H).transpose(1, 0, 2)).reshape(3 * H, H), ob1=inputs["out_b1"].astype(f)[:, None],
        o2_t=inputs["out_w2"].astype(f).T.copy(), ob2=inputs["out_b2"].astype(f)[:, None],
        o3_t=inputs["out_w3"].astype(f).T.copy(), ob3=inputs["out_b3"].astype(f)[:, None],
    )

    in_maps = []
    for c in range(bs.shape[0] // BS_):
        sl = slice(c * BS_, (c + 1) * BS_)
        s = bs[sl]                                    # [BS, T, SD]
        m = bm[sl]                                    # [BS, T]
        T_ = s.shape[1]
        # t-major token order: tok = t*BS + b
        series_tm = np.ascontiguousarray(s.transpose(2, 1, 0).reshape(SD, T_ * BS_))
        mb_row = np.ascontiguousarray(
            (MASK_BIG * (1.0 - m.T)).reshape(1, T_ * BS_))
        delta = m.copy()
        delta[:, :-1] -= m[:, 1:]
        delta_row = np.ascontiguousarray(delta.T.reshape(1, T_ * BS_))
        bff = ml_dtypes.bfloat16
        im = dict(shared)
        im.update(series_t=series_tm.astype(bff), mb_row=mb_row.astype(bff),
                  delta_row=delta_row.astype(bff),
                  feat_t=bf[sl].T.copy().astype(f))
        in_maps.append(im)
    return in_maps


_CACHE = {}


def kernel(**inputs):
    if "nc" not in _CACHE:
        nc = bacc.Bacc(None, target_bir_lowering=False)
        build(nc)
        _CACHE["nc"] = nc
    nc = _CACHE["nc"]
    in_maps = host_prep(inputs)
    res = run_bass_kernel_spmd(nc, in_maps, core_ids=list(range(NCORES)))
    outs = [r["out"].reshape(BS) for r in res.results]
    return np.concatenate(outs).reshape(B, 1).astype(np.float32)


if __name__ == "__main__":
    sys.path.insert(0, "/root/problem")
    import reference
    inputs = {k: np.asarray(v) for k, v in reference.setup_inputs().items()}
    out = kernel(**inputs)
    exp = np.asarray(reference.reference(**inputs))
    err = np.abs(out - exp).max() / (np.abs(exp).max() + 1e-9)
    print("max out", np.abs(out).max(), "rel err", err)


# revision 18
# speedup vs baseline: 1.2395x; 1.2395x over previous
"""Trainium2 Bass kernel for nn_Amodel_20933670600894 (ragged bi-GRU + MLP).

Data parallel over 8 cores (32 sequences each). Per core:
  Phase A: x1 = LayerNorm(series @ w_in + b_in)  -- LN done via centered
           weights (mean fold) + variance from a ones-matmul of squares;
           ln_g/ln_b folded into the gate matmul weights.
           gates_x = x1n @ (wi*ln_g).T + biases, with +30 bias folded into
           the z-gate wherever mask==0 so the time scan needs no mask.
           x_last (x1 at t=len-1) accumulated via a delta one-hot matmul.
  Phase B: 1024-step masked GRU scan, h kept as [128(h), 32(batch)] in SBUF.
  Phase C: backward GRU cell at last step, feature MLP, fusion head.
"""
import sys, os
sys.path.insert(0, "/opt/trn_rl_repo")

import numpy as np
import ml_dtypes
from contextlib import ExitStack

import concourse.bass as bass
import concourse.mybir as mybir
import concourse.tile as tile
from concourse import bacc
from concourse.bass_utils import run_bass_kernel_spmd

AF = mybir.ActivationFunctionType
ALU = mybir.AluOpType
F32 = mybir.dt.float32
BF16 = mybir.dt.bfloat16

B, T, SD, FD, H, NHID = 256, 1024, 64, 128, 128, 3
NCORES = 8
BS = B // NCORES          # 32 sequences per core
EPS = 1e-5
MASK_BIG = 30.0


def do_c_flag(p):
    return 'C' in p


def build(nc, T_=T, BS_=BS, CH_A=512, CH_S=64, phases='ABC'):
    """Build the per-core program. Token index = t*BS_ + b (t-major)."""
    NTOK = T_ * BS_
    CH_A = min(CH_A, NTOK, CH_S * BS_)
    n_tiles = NTOK // CH_A
    n_chunks = T_ // CH_S

    with tile.TileContext(nc) as tc:
        ctx = ExitStack()
        dram = ctx.enter_context(tc.tile_pool(name="dram", bufs=1, space="DRAM"))

        def din(name, shape):
            return dram.tile(shape, F32, kind="ExternalInput", name=name,
                             uniquify=False)

        series_t = dram.tile([SD, NTOK], BF16, kind="ExternalInput",
                              name="series_t", uniquify=False)
        mb_row = dram.tile([1, NTOK], BF16, kind="ExternalInput",
                            name="mb_row", uniquify=False)
        delta_row = dram.tile([1, NTOK], BF16, kind="ExternalInput",
                               name="delta_row", uniquify=False)
        w1_ext = din("w1_ext", [SD, H])              # W_centered
        b_ct = din("b_ct", [1, H])                   # b_centered
        wi_s = din("wi_s", [H, 3 * H])               # (wi * ln_g).T fwd
        bi_tot = din("bi_tot", [H, 3])               # per-gate bias totals fwd
        wh_t = din("wh_t", [H, 4 * H])               # [Wr,Wz,Wn,-Wz].T
        bhn = din("bhn", [H, 1])                     # bh_f n-slice
        wib_s = din("wib_s", [H, 3 * H])             # (wi_b * ln_g).T bwd
        bib_tot = din("bib_tot", [H, 3])             # per-gate bias totals bwd
        bhbn = din("bhbn", [H, 1])                   # bh_b n-slice
        feat_t = din("feat_t", [FD, BS_])            # feature transposed
        w0_t = din("w0_t", [FD, H])                  # feat_w0.T
        mlp_s = din("mlp_s", [H, NHID])              # bn scale per layer
        mlp_b = din("mlp_b", [H, NHID])              # bn shift per layer
        hw_t = din("hw_t", [H, (NHID - 1) * H])      # hid_w[i].T stacked
        o1_t = din("o1_t", [3 * H, H])               # out_w1.T
        ob1 = din("ob1", [H, 1])
        o2_t = din("o2_t", [H, H])                   # out_w2.T
        ob2 = din("ob2", [H, 1])
        o3_t = din("o3_t", [H, 1])                   # out_w3.T
        ob3 = din("ob3", [1, 1])
        out = dram.tile([1, BS_], F32, kind="ExternalOutput", name="out",
                        uniquify=False)


        const = ctx.enter_context(tc.tile_pool(name="const", bufs=1))
        # small constant tiles
        ones_div = const.tile([H, H], BF16)     # 1/H everywhere (var reduce)
        nc.vector.memset(ones_div[:], 1.0 / H)
        one_row = const.tile([1, H], BF16)      # broadcast row of ones
        nc.vector.memset(one_row[:], 1.0)
        eps_col = const.tile([H, 1], F32)
        nc.vector.memset(eps_col[:], EPS)

        _ld = [0]

        def load(pool, src, shape=None, name=None):
            _ld[0] += 1
            t_ = pool.tile(shape or src.shape, F32,
                           name=name or f"ld{_ld[0]}", tag=f"ldt{_ld[0]}")
            nc.sync.dma_start(t_[:], src[:])
            return t_

        def load_bf(pool, src, name):
            f32t = pool.tile(src.shape, F32, name=name + "_f", tag=name + "_f")
            nc.sync.dma_start(f32t[:], src[:])
            bft = pool.tile(src.shape, BF16, name=name, tag=name)
            nc.vector.tensor_copy(bft[:], f32t[:])
            return bft

        w1e_sb = load_bf(const, w1_ext, "w1e")  # [64, 128] bf16
        bct_sb = load_bf(const, b_ct, "bct")    # [1, 128] bf16
        wis_sb = load_bf(const, wi_s, "wis")    # [128, 384] bf16
        bit_sb = load(const, bi_tot)            # [128, 3]
        wht_sb = load_bf(const, wh_t, "wht")    # [128, 384] bf16
        bhn_sb = load(const, bhn)
        from concourse.masks import make_identity
        ident = const.tile([H, H], BF16, name="ident")
        make_identity(nc, ident[:])
        ones_ca = const.tile([1, CH_A], BF16, name="ones_ca")
        nc.vector.memset(ones_ca[:], 1.0)

        xacc = const.tile([H, CH_A], F32, name="xacc")
        nc.vector.memset(xacc[:], 0.0)

        # ---------------- Phases A+B interleaved: gate precompute feeds the
        # scan through an SBUF ring; Phase A work fills the scan's idle slots.
        ctx_a = ExitStack()
        pa = ctx_a.enter_context(tc.tile_pool(name="pa", bufs=2))
        pp_a = ctx_a.enter_context(tc.tile_pool(name="pp_a", bufs=1, space="PSUM"))
        pp_b = ctx_a.enter_context(tc.tile_pool(name="pp_b", bufs=1, space="PSUM"))
        pp_g = ctx_a.enter_context(tc.tile_pool(name="pp_g", bufs=1, space="PSUM"))
        pp_d = ctx_a.enter_context(tc.tile_pool(name="pp_d", bufs=1, space="PSUM"))
        ps = ctx_a.enter_context(tc.tile_pool(name="ps", bufs=2))
        pp_s = ctx_a.enter_context(tc.tile_pool(name="pp_s", bufs=2, space="PSUM"))

        h = const.tile([H, BS_], BF16, name="h")
        nc.vector.memset(h[:], 0.0)

        TPC = CH_S * BS_              # tokens per scan chunk
        apc = max(1, TPC // CH_A)     # A-tiles per scan chunk
        assert apc * CH_A == TPC or TPC < CH_A

        ring = []                     # (crz, cn) per chunk, pool-rotated

        def emit_a_chunk(c):
            """Phase A for scan chunk c: produce its gx ring tiles."""
            crz = ps.tile([H, CH_S * 3 * BS_], BF16, tag="crz")
            cn = ps.tile([H, CH_S * BS_], BF16, tag="cn")
            ring.append((crz, cn))
            for a in range(apc):
                i = c * apc + a
                S = slice(i * CH_A, (i + 1) * CH_A)
                s_t = pa.tile([SD, CH_A], BF16, tag="s_t")
                nc.sync.dma_start(s_t[:], series_t[:, S])
                mb_t = pa.tile([1, CH_A], BF16, tag="mb_t")
                nc.sync.dma_start(mb_t[:], mb_row[:, S])
                dl_t = pa.tile([1, CH_A], BF16, tag="dl_t")
                nc.sync.dma_start(dl_t[:], delta_row[:, S])
                x1c = pp_a.tile([H, CH_A], F32, tag="x1c")
                nc.tensor.matmul(x1c[:], w1e_sb[:], s_t[:], start=True, stop=False)
                nc.tensor.matmul(x1c[:], bct_sb[:], ones_ca[:], start=False,
                                 stop=True)
                x1s = pa.tile([H, CH_A], F32, tag="x1s")
                nc.vector.tensor_copy(x1s[:], x1c[:])
                sq = pa.tile([H, CH_A], BF16, tag="sq")
                nc.vector.tensor_mul(sq[:], x1s[:], x1s[:])
                var = pp_b.tile([H, CH_A], F32, tag="var")
                nc.tensor.matmul(var[:], ones_div[:], sq[:], start=True, stop=True)
                lnv = pa.tile([H, CH_A], F32, tag="lnv")
                nc.scalar.activation(lnv[:], var[:], AF.Ln, bias=eps_col[:, 0:1])
                rstd = pa.tile([H, CH_A], F32, tag="rstd")
                nc.scalar.activation(rstd[:], lnv[:], AF.Exp, scale=-0.5)
                x1n = pa.tile([H, CH_A], BF16, tag="x1n")
                nc.vector.tensor_mul(x1n[:], x1s[:], rstd[:])

                g_r = pp_g.tile([H, CH_A], F32, tag="g_r")
                g_z = pp_g.tile([H, CH_A], F32, tag="g_z")
                g_n = pp_g.tile([H, CH_A], F32, tag="g_n")
                nc.tensor.matmul(g_r[:], wis_sb[:, 0:H], x1n[:], start=True,
                                 stop=True)
                nc.tensor.matmul(g_z[:], wis_sb[:, H:2 * H], x1n[:], start=True,
                                 stop=False)
                nc.tensor.matmul(g_z[:], one_row[:], mb_t[:], start=False,
                                 stop=True)
                nc.tensor.matmul(g_n[:], wis_sb[:, 2 * H:3 * H], x1n[:],
                                 start=True, stop=True)
                nt = CH_A // BS_
                # evac straight into the ring tiles ([r,z,zneg] per step)
                rview = crz[:, 3 * a * CH_A:3 * (a + 1) * CH_A].rearrange(
                    "h (t three b) -> h (t three) b", three=3, b=BS_)
                dst_r = rview[:, 0::3, :]
                dst_z = rview[:, 1::3, :]
                dst_zn = rview[:, 2::3, :]
                nc.vector.tensor_scalar(dst_r, g_r[:].rearrange(
                    "h (t b) -> h t b", b=BS_), bit_sb[:, 0:1], None, op0=ALU.add)
                nc.vector.tensor_scalar(dst_z, g_z[:].rearrange(
                    "h (t b) -> h t b", b=BS_), bit_sb[:, 1:2], None, op0=ALU.add)
                nc.vector.tensor_scalar(dst_zn, g_z[:].rearrange(
                    "h (t b) -> h t b", b=BS_), bit_sb[:, 1:2], -1.0,
                    op0=ALU.add, op1=ALU.mult)
                nc.vector.tensor_scalar(cn[:, a * CH_A:(a + 1) * CH_A], g_n[:],
                                        bit_sb[:, 2:3], None, op0=ALU.add)

                db = pp_d.tile([H, CH_A], F32, tag="db")
                nc.tensor.matmul(db[:], one_row[:], dl_t[:], start=True,
                                 stop=True)
                tmp = pa.tile([H, CH_A], F32, tag="xtmp")
                nc.vector.tensor_mul(tmp[:], x1n[:], db[:])
                nc.vector.tensor_add(xacc[:], xacc[:], tmp[:])

        def emit_scan_chunk(c):
            crz, cn = ring[c]
            for j in range(CH_S):
                g = pp_s.tile([H, 4 * BS_], F32, tag="g")
                nc.tensor.matmul(g[:, 0:3 * BS_], ident[:],
                                 crz[:, j * 3 * BS_:(j + 1) * 3 * BS_],
                                 start=True, stop=False)
                nc.tensor.matmul(g[:, 0:BS_], wht_sb[:, 0:H], h[:],
                                 start=False, stop=True)
                nc.tensor.matmul(g[:, BS_:2 * BS_], wht_sb[:, H:2 * H], h[:],
                                 start=False, stop=True, skip_group_check=True)
                nc.tensor.matmul(g[:, 2 * BS_:3 * BS_], wht_sb[:, 3 * H:4 * H],
                                 h[:], start=False, stop=True,
                                 skip_group_check=True)
                nc.tensor.matmul(g[:, 3 * BS_:4 * BS_], wht_sb[:, 2 * H:3 * H],
                                 h[:], start=True, stop=True)
                rzz = ps.tile([H, 3 * BS_], F32, tag="rzz")
                nc.scalar.activation(rzz[:], g[:, 0:3 * BS_], AF.Sigmoid)
                e2 = ps.tile([H, BS_], F32, tag="e2")
                nc.vector.scalar_tensor_tensor(
                    e2[:], g[:, 3 * BS_:4 * BS_], bhn_sb[:, 0:1], rzz[:, 0:BS_],
                    op0=ALU.add, op1=ALU.mult)
                t2 = ps.tile([H, BS_], F32, tag="t2")
                nc.vector.tensor_add(t2[:], e2[:],
                                     cn[:, j * BS_:(j + 1) * BS_])
                u_ = ps.tile([H, BS_], F32, tag="u_")
                nc.vector.tensor_mul(u_[:], rzz[:, BS_:2 * BS_], h[:])
                s_ = ps.tile([H, BS_], F32, tag="s_")
                nc.scalar.activation(s_[:], t2[:], AF.Sigmoid, scale=2.0)
                v_ = ps.tile([H, BS_], F32, tag="v_")
                nc.vector.scalar_tensor_tensor(v_[:], s_[:], 0.5,
                                               rzz[:, 2 * BS_:3 * BS_],
                                               op0=ALU.subtract, op1=ALU.mult)
                nc.vector.scalar_tensor_tensor(h[:], v_[:], 2.0, u_[:],
                                               op0=ALU.mult, op1=ALU.add)

        if 'A' in phases:
            emit_a_chunk(0)
            for c in range(n_chunks):
                if c + 1 < n_chunks:
                    emit_a_chunk(c + 1)
                if 'B' in phases:
                    emit_scan_chunk(c)

        # reduce xacc [H, CH_A] -> x_last [H, BS_] (tree over the t groups)
        width = CH_A if 'A' in phases else BS_
        while width > BS_:
            half = width // 2
            nc.vector.tensor_add(xacc[:, 0:half], xacc[:, 0:half],
                                 xacc[:, half:width])
            width = half
        x_last = xacc[:, 0:BS_]

        ctx_a.close()

        # ---------------- Phase C: backward cell, MLP, head ----------------
        pc = ctx.enter_context(tc.tile_pool(name="pc", bufs=1))
        pp_c = ctx.enter_context(tc.tile_pool(name="pp_c", bufs=1, space="PSUM"))
        wibs_sb = load_bf(pc, wib_s, "wibs")
        bibt_sb = load(pc, bib_tot)
        bhbn_sb = load(pc, bhbn)

        xl_bf = pc.tile([H, BS_], BF16, name="xl_bf")
        nc.vector.tensor_copy(xl_bf[:], x_last)
        gb = pp_c.tile([H, 3 * BS_], F32, tag="gb")
        for s in range(3):
            nc.tensor.matmul(gb[:, s * BS_:(s + 1) * BS_],
                             wibs_sb[:, s * H:(s + 1) * H], xl_bf[:],
                             start=True, stop=True)
        rb = pc.tile([H, BS_], F32, name="rb")
        nc.scalar.activation(rb[:], gb[:, 0:BS_], AF.Sigmoid,
                             bias=bibt_sb[:, 0:1])
        zb = pc.tile([H, BS_], F32, name="zb")
        nc.scalar.activation(zb[:], gb[:, BS_:2 * BS_], AF.Sigmoid,
                             bias=bibt_sb[:, 1:2])
        ub = pc.tile([H, BS_], F32, name="ub")
        nc.vector.tensor_scalar_mul(ub[:], rb[:], bhbn_sb[:, 0:1])
        tb = pc.tile([H, BS_], F32, name="tb")
        nc.vector.scalar_tensor_tensor(tb[:], gb[:, 2 * BS_:3 * BS_],
                                       bibt_sb[:, 2:3], ub[:],
                                       op0=ALU.add, op1=ALU.add)
        nb = pc.tile([H, BS_], F32, name="nb")
        nc.scalar.activation(nb[:], tb[:], AF.Tanh)
        vb = pc.tile([H, BS_], F32, name="vb")
        nc.vector.tensor_mul(vb[:], zb[:], nb[:])
        h_bwd = pc.tile([H, BS_], BF16, name="h_bwd")
        nc.vector.tensor_sub(h_bwd[:], nb[:], vb[:])

        # feature MLP
        featt_sb = load_bf(pc, feat_t, "featt")
        w0t_sb = load_bf(pc, w0_t, "w0t")
        mlps_sb = load(pc, mlp_s)
        mlpb_sb = load(pc, mlp_b)
        hwt_sb = load_bf(pc, hw_t, "hwt")
        x2 = featt_sb
        wts = [w0t_sb[:]] + [hwt_sb[:, i * H:(i + 1) * H] for i in range(NHID - 1)]
        for li in range(NHID):
            pm = pp_c.tile([H, BS_], F32, tag="pc")
            nc.tensor.matmul(pm[:], wts[li], x2[:], start=True, stop=True)
            x2n = pc.tile([H, BS_], BF16, name=f"x2_{li}")
            nc.scalar.activation(x2n[:], pm[:], AF.Lrelu,
                                 bias=mlpb_sb[:, li:li + 1],
                                 scale=mlps_sb[:, li:li + 1], alpha=0.01)
            x2 = x2n

        # head
        o1t_f = pc.tile([H, 3, H], F32, name="o1t_f")
        nc.sync.dma_start(o1t_f[:], o1_t[:])
        o1t_sb = pc.tile([H, 3, H], BF16, name="o1t")
        nc.vector.tensor_copy(o1t_sb[:], o1t_f[:])
        ob1_sb = load(pc, ob1)
        o2t_sb = load_bf(pc, o2_t, "o2t")
        ob2_sb = load(pc, ob2)
        o3t_sb = load_bf(pc, o3_t, "o3t")
        ob3_sb = load(pc, ob3)

        p1 = pp_c.tile([H, BS_], F32, tag="pc")
        nc.tensor.matmul(p1[:], o1t_sb[:, 0, :], h[:], start=True, stop=False)
        nc.tensor.matmul(p1[:], o1t_sb[:, 1, :], h_bwd[:], start=False, stop=False)
        nc.tensor.matmul(p1[:], o1t_sb[:, 2, :], x2[:], start=False, stop=True)
        y1 = pc.tile([H, BS_], BF16, name="y1")
        nc.scalar.activation(y1[:], p1[:], AF.Lrelu, bias=ob1_sb[:, 0:1],
                             alpha=0.01)
        p2 = pp_c.tile([H, BS_], F32, tag="pc")
        nc.tensor.matmul(p2[:], o2t_sb[:], y1[:], start=True, stop=True)
        y2 = pc.tile([H, BS_], BF16, name="y2")
        nc.scalar.activation(y2[:], p2[:], AF.Lrelu, bias=ob2_sb[:, 0:1],
                             alpha=0.01)
        p3 = pp_c.tile([1, BS_], F32, tag="pc3")
        nc.tensor.matmul(p3[:], o3t_sb[:], y2[:], start=True, stop=True)
        y3 = pc.tile([1, BS_], F32, name="y3")
        nc.scalar.activation(y3[:], p3[:], AF.Sigmoid, bias=ob3_sb[0:1, 0:1])
        nc.sync.dma_start(out[:], y3[:])

        ctx.close()
    nc.compile()
    return nc


def host_prep(inputs, BS_=BS):
    """Per-core input dicts (layout prep only)."""
    f = np.float32
    bs = inputs["batch_series"].astype(f)
    bm = inputs["batch_mask"].astype(f)
    bf = inputs["batch_feature"].astype(f)
    w_in, b_in = inputs["w_in"].astype(f), inputs["b_in"].astype(f)
    ln_g, ln_b = inputs["ln_g"].astype(f), inputs["ln_b"].astype(f)
    wi_f, wh_f = inputs["gru_wi_f"].astype(f), inputs["gru_wh_f"].astype(f)
    bi_f, bh_f = inputs["gru_bi_f"].astype(f), inputs["gru_bh_f"].astype(f)
    wi_b = inputs["gru_wi_b"].astype(f)
    bi_b, bh_b = inputs["gru_bi_b"].astype(f), inputs["gru_bh_b"].astype(f)

    # LN folds
    w_ct = (w_in - w_in.mean(0, keepdims=True)).T.copy()        # [SD, H]
    b_ct = (b_in - b_in.mean())[None, :]                        # [1, H]
    w1_ext = w_ct.astype(f)                                     # [SD, H]
    wi_s = (wi_f * ln_g[None, :]).T.copy().astype(f)            # [H, 3H]
    wib_s = (wi_b * ln_g[None, :]).T.copy().astype(f)
    lnb_f = wi_f @ ln_b                                          # [3H]
    lnb_b = wi_b @ ln_b
    bt_f = bi_f + lnb_f
    bt_f[0:2 * H] += bh_f[0:2 * H]
    bi_tot = np.stack([bt_f[0:H], bt_f[H:2 * H], bt_f[2 * H:3 * H]], 1).astype(f)
    bt_b = bi_b + lnb_b
    bt_b[0:2 * H] += bh_b[0:2 * H]
    bib_tot = np.stack([bt_b[0:H], bt_b[H:2 * H], bt_b[2 * H:3 * H]], 1).astype(f)

    bn_scale = 1.0 / np.sqrt(1.0 + EPS)
    mlp_s = np.stack([inputs["bn0_g"].astype(f) * bn_scale] +
                     [inputs["hbn_g"][i].astype(f) * bn_scale
                      for i in range(NHID - 1)], 1).astype(f)
    mlp_b = np.stack(
        [inputs["feat_b0"].astype(f) * bn_scale * inputs["bn0_g"].astype(f)
         + inputs["bn0_b"].astype(f)] +
        [inputs["hid_b"][i].astype(f) * bn_scale * inputs["hbn_g"][i].astype(f)
         + inputs["hbn_b"][i].astype(f) for i in range(NHID - 1)],
        1).astype(f)
    hw_t = np.concatenate([inputs["hid_w"][i].astype(f).T
                           for i in range(NHID - 1)], 1).astype(f)

    shared = dict(
        w1_ext=w1_ext, b_ct=np.ascontiguousarray(b_ct).astype(f), wi_s=wi_s,
        bi_tot=bi_tot,
        wh_t=np.concatenate([wh_f.T, -wh_f.T[:, H:2 * H]], 1).copy().astype(f),
        bhn=bh_f[2 * H:3 * H, None].astype(f),
        wib_s=wib_s, bib_tot=bib_tot,
        bhbn=bh_b[2 * H:3 * H, None].astype(f),
        w0_t=inputs["feat_w0"].astype(f).T.copy(),
        mlp_s=mlp_s, mlp_b=mlp_b, hw_t=hw_t,
        o1_t=np.ascontiguousarray(inputs["out_w1"].astype(f).T.reshape(3, H, H).transpose(1, 0, 2)).reshape(3 * H, H), ob1=inputs["out_b1"].astype(f)[:, None],
        o2_t=inputs["out_w2"].astype(f).T.copy(), ob2=inputs["out_b2"].astype(f)[:, None],
        o3_t=inputs["out_w3"].astype(f).T.copy(), ob3=inputs["out_b3"].astype(f)[:, None],
    )

    in_maps = []
    for c in range(bs.shape[0] // BS_):
        sl = slice(c * BS_, (c + 1) * BS_)
        s = bs[sl]                                    # [BS, T, SD]
        m = bm[sl]                                    # [BS, T]
        T_ = s.shape[1]
        # t-major token order: tok = t*BS + b
        series_tm = np.ascontiguousarray(s.transpose(2, 1, 0).reshape(SD, T_ * BS_))
        mb_row = np.ascontiguousarray(
            (MASK_BIG * (1.0 - m.T)).reshape(1, T_ * BS_))
        delta = m.copy()
        delta[:, :-1] -= m[:, 1:]
        delta_row = np.ascontiguousarray(delta.T.reshape(1, T_ * BS_))
        bff = ml_dtypes.bfloat16
        im = dict(shared)
        im.update(series_t=series_tm.astype(bff), mb_row=mb_row.astype(bff),
                  delta_row=delta_row.astype(bff),
                  feat_t=bf[sl].T.copy().astype(f))
        in_maps.append(im)
    return in_maps


_CACHE = {}


def kernel(**inputs):
    if "nc" not in _CACHE:
        nc = bacc.Bacc(None, target_bir_lowering=False)
        build(nc)
        _CACHE["nc"] = nc
    nc = _CACHE["nc"]
    in_maps = host_prep(inputs)
    res = run_bass_kernel_spmd(nc, in_maps, core_ids=list(range(NCORES)))
    outs = [r["out"].reshape(BS) for r in res.results]
    return np.concatenate(outs).reshape(B, 1).astype(np.float32)


if __name__ == "__main__":
    sys.path.insert(0, "/root/problem")
    import reference
    inputs = {k: np.asarray(v) for k, v in reference.setup_inputs().items()}
    out = kernel(**inputs)
    exp = np.asarray(reference.reference(**inputs))
    err = np.abs(out - exp).max() / (np.abs(exp).max() + 1e-9)
    print("max out", np.abs(out).max(), "rel err", err)
